# revision 1
# baseline (speedup 1.0000x reference)
"""Single-head attention (B=4, S=2048, D=E=1024) on 8 trn2 NeuronCores.

Sharding: data-parallel over (batch, q-half) -> 8 shards. Each core gets a
1024-row q shard plus the full 2048 keys of its batch; K/V projections are
recomputed on both cores of a batch pair (25% extra flops, zero collectives).

Per-core math (all "T" tensors are token-transposed on the host so that the
contraction dim lands on SBUF partitions; no on-device transposes needed):
  qp^T [E,q]   = (lhsT=wq[D,E], rhs=qT[D,q]) * (1/sqrt E) + bq/sqrt(E)
  kp^T [E,k]   = (lhsT=wk, rhs=kT) + bk
  vp   [k,E]   = (lhsT=vT[D,k], rhs=wv[D,E]) + bv
  lgT  [k,q]   = (lhsT=kp^T slice, rhs=qp^T)            (scale folded into qp)
  expT [k,q]   = Exp(lgT + mask*NEG)                    (ACT, per-partition bias)
  s    [.,q]   = ones-matmul over expT                  (softmax sum; no max-sub:
                                                         logits ~ N(0,1), safe)
  ctx^T[E,q]   = (lhsT=vp slice, rhs=expT) * recip(s)
  out  [q,D]   = (lhsT=ctx^T slice, rhs=ow[E,D]) + ob
All matmuls run as float32r (full PE rate at N>=256), fp32 data + accumulate.
Pool lifetimes follow strict LIFO (Tile pool-stack requirement).
"""

import os
import numpy as np

P = 128
NEG = -1.0e9


def build_nc(D=1024, E=1024, SK=2048, QSH=1024, QB=512):
    """Build the per-core Bass module (SPMD; same program on all cores)."""
    import concourse.bass as bass
    import concourse.mybir as mybir
    import concourse.tile as tile
    from concourse import bacc

    f32 = mybir.dt.float32
    f32r = mybir.dt.float32r
    AF = mybir.ActivationFunctionType

    DT = D // P          # contraction tiles over model dim
    ET = E // P          # enc tiles
    KT = SK // P         # key tiles
    NQB = QSH // QB      # q blocks
    KNB = min(512, SK)   # key free-dim block for kp
    ENB = min(512, E)    # E free-dim block for vp
    DNB = min(512, D)    # model free-dim block for out
    DTH = max(1, DT // 2)  # split-K half for kp streaming
    ISCALE = 1.0 / float(np.sqrt(E))

    nc = bacc.Bacc(trn_type="TRN2")

    # ---- I/O ----
    qT = nc.dram_tensor("qT", [D, QSH], f32r, kind="ExternalInput")[:, :]
    kT = nc.dram_tensor("kT", [D, SK], f32r, kind="ExternalInput")[:, :]
    vT = nc.dram_tensor("vT", [D, SK], f32r, kind="ExternalInput")[:, :]
    mask_cols = nc.dram_tensor("mask_cols", [P, KT], f32, kind="ExternalInput")[:, :]
    ones_d = nc.dram_tensor("ones_d", [P, P], f32r, kind="ExternalInput")[:, :]
    wq = nc.dram_tensor("wq", [D, E], f32r, kind="ExternalInput")[:, :]
    wk = nc.dram_tensor("wk", [D, E], f32r, kind="ExternalInput")[:, :]
    wv = nc.dram_tensor("wv", [D, E], f32r, kind="ExternalInput")[:, :]
    ow = nc.dram_tensor("ow", [E, D], f32r, kind="ExternalInput")[:, :]
    bq_col = nc.dram_tensor("bq_col", [P, ET], f32, kind="ExternalInput")[:, :]
    bk_col = nc.dram_tensor("bk_col", [P, ET], f32, kind="ExternalInput")[:, :]
    bv_bc = nc.dram_tensor("bv_bc", [P, E], f32, kind="ExternalInput")[:, :]
    ob_bc = nc.dram_tensor("ob_bc", [P, D], f32, kind="ExternalInput")[:, :]
    out = nc.dram_tensor("out", [QSH, D], f32, kind="ExternalOutput")[:, :]

    qT_r = qT.rearrange("(t p) n -> p t n", p=P)   # [128, DT, QSH]
    kT_r = kT.rearrange("(t p) n -> p t n", p=P)
    vT_r = vT.rearrange("(t p) n -> p t n", p=P)
    wq_r = wq.rearrange("(t p) n -> p t n", p=P)   # [128, DT, E]
    wk_r = wk.rearrange("(t p) n -> p t n", p=P)
    wv_r = wv.rearrange("(t p) n -> p t n", p=P)
    ow_r = ow.rearrange("(t p) n -> p t n", p=P)   # [128, ET, D]

    def mm(ps, lhsT, rhs, start, stop):
        nc.tensor.matmul(ps, lhsT, rhs, start=start, stop=stop)

    with tile.TileContext(nc) as tc:
        # ---- persistent smalls (incl. per-qb softmax reciprocal + out bias) ----
        smalls_cm = tc.tile_pool(name="smalls", bufs=1)
        smalls = smalls_cm.__enter__()
        ones_t = smalls.tile([P, P], f32r, name="ones")
        nc.gpsimd.dma_start(ones_t[:], ones_d)
        mask_t = smalls.tile([P, KT], f32, name="maskc")
        nc.gpsimd.dma_start(mask_t[:], mask_cols)
        nc.scalar.mul(mask_t[:], mask_t[:], NEG)
        bq_t = smalls.tile([P, ET], f32, name="bqc")
        nc.gpsimd.dma_start(bq_t[:], bq_col)
        nc.scalar.mul(bq_t[:], bq_t[:], ISCALE)
        bk_t = smalls.tile([P, ET], f32, name="bkc")
        nc.gpsimd.dma_start(bk_t[:], bk_col)
        recip_ts = [smalls.tile([P, QB], f32, name=f"recip{i}")
                    for i in range(NQB)]

        dram_cm = tc.tile_pool(name="dramscratch", bufs=1, space="DRAM")
        dram_pool = dram_cm.__enter__()

        # ---- phase VP (first: vp outlives kp): vp [SK, E] + bv ----
        vp_cm = tc.tile_pool(name="vp", bufs=1)
        vp_pool = vp_cm.__enter__()
        vp = vp_pool.tile([P, KT, E], f32r, name="vp")
        with tc.tile_pool(name="vp_w", bufs=1) as phw, \
             tc.tile_pool(name="vp_ph", bufs=3) as ph, \
             tc.tile_pool(name="vp_ps", bufs=4, space="PSUM") as php:
            wv_t = phw.tile([P, DT, E], f32r, name="wv_t")
            NH = 2 if E >= 512 else 1
            for h in range(NH):
                for t in range(DT):
                    nc.sync.dma_start(wv_t[:, t, h * E // NH:(h + 1) * E // NH],
                                      wv_r[:, t, h * E // NH:(h + 1) * E // NH])
            bv_t = phw.tile([P, E], f32, name="bv_t")
            nc.sync.dma_start(bv_t[:], bv_bc)
            for m in range(KT):
                lhs_t = ph.tile([P, DT, P], f32r, tag="vT_s", name=f"vT_{m}")
                hh = max(1, DT // 2)
                nc.scalar.dma_start(lhs_t[:, :hh, :],
                                    vT_r[:, :hh, m * P:(m + 1) * P])
                nc.gpsimd.dma_start(lhs_t[:, hh:, :],
                                    vT_r[:, hh:, m * P:(m + 1) * P])
                for n in range(E // ENB):
                    ps = php.tile([P, ENB], f32, tag="ps", name=f"vpps_{m}_{n}")
                    for t in range(DT):
                        mm(ps[:], lhs_t[:, t, :],
                           wv_t[:, t, n * ENB:(n + 1) * ENB],
                           t == 0, t == DT - 1)
                    nc.vector.tensor_add(vp[:, m, n * ENB:(n + 1) * ENB], ps[:],
                                         bv_t[:, n * ENB:(n + 1) * ENB])

        # ---- phase KP: kp^T [E, SK] + bk (kT streamed in split-K halves) ----
        kp_cm = tc.tile_pool(name="kp", bufs=1)
        kp_pool = kp_cm.__enter__()
        kp = kp_pool.tile([P, ET, SK], f32r, name="kp")
        with tc.tile_pool(name="kp_w", bufs=1) as phw, \
             tc.tile_pool(name="kp_ph", bufs=3) as ph, \
             tc.tile_pool(name="kp_ps", bufs=1, space="PSUM") as php:
            wk_t = phw.tile([P, DT, E], f32r, name="wk_t")
            for h in range(2):
                for t in range(DT):
                    eng = nc.sync if t % 2 == 0 else nc.scalar
                    eng.dma_start(wk_t[:, t, h * E // 2:(h + 1) * E // 2],
                                  wk_r[:, t, h * E // 2:(h + 1) * E // 2])
            for n in range(SK // KNB):
                pss = [php.tile([P, KNB], f32, tag=f"ps{m}", name=f"kpps_{n}_{m}")
                       for m in range(ET)]
                for th in range(DT // DTH):
                    rhs_t = ph.tile([P, DTH, KNB], f32r, tag="kT_s",
                                    name=f"kT_{n}_{th}")
                    for ti in range(DTH):
                        t = th * DTH + ti
                        eng = nc.gpsimd
                        eng.dma_start(rhs_t[:, ti, :],
                                      kT_r[:, t, n * KNB:(n + 1) * KNB])
                    for m in range(ET):
                        for ti in range(DTH):
                            t = th * DTH + ti
                            mm(pss[m][:], wk_t[:, t, m * P:(m + 1) * P],
                               rhs_t[:, ti, :], t == 0, t == DT - 1)
                for m in range(ET):
                    nc.scalar.activation(kp[:, m, n * KNB:(n + 1) * KNB],
                                         pss[m][:], AF.Identity,
                                         bias=bk_t[:, m:m + 1])

        # ---- attention per q-block ----
        ctx_bounce = []
        ctx_last = None
        ctx_last_cm = None

        for qb in range(NQB):
            q0 = qb * QB
            last_qb = qb == NQB - 1

            exp_cm = tc.tile_pool(name=f"exp{qb}", bufs=1)
            exp_pool = exp_cm.__enter__()
            expT = exp_pool.tile([P, KT, QB], f32r, name=f"exp{qb}")

            # -- prologue: qp^T for this q block --
            qp_cm = tc.tile_pool(name=f"qp{qb}", bufs=1)
            qp_pool = qp_cm.__enter__()
            qp = qp_pool.tile([P, ET, QB], f32r, name=f"qp{qb}")
            with tc.tile_pool(name=f"qpro{qb}", bufs=2) as ph, \
                 tc.tile_pool(name=f"qpro_ps{qb}", bufs=1, space="PSUM") as php:
                pss = [php.tile([P, QB], f32, tag=f"ps{m}", name=f"qpps{qb}_{m}")
                       for m in range(ET)]
                for t in range(DT):
                    wq_t = ph.tile([P, E], f32r, tag="wq_s", name=f"wq{qb}_{t}")
                    for h in range(2):
                        eng = nc.sync if h == 0 else nc.scalar
                        eng.dma_start(wq_t[:, h * E // 2:(h + 1) * E // 2],
                                      wq_r[:, t, h * E // 2:(h + 1) * E // 2])
                    qt_t = ph.tile([P, QB], f32r, tag="qT_s", name=f"qt{qb}_{t}")
                    nc.scalar.dma_start(qt_t[:], qT_r[:, t, q0:q0 + QB])
                    for m in range(ET):
                        mm(pss[m][:], wq_t[:, m * P:(m + 1) * P], qt_t[:],
                           t == 0, t == DT - 1)
                for m in range(ET):
                    nc.scalar.activation(qp[:, m, :], pss[m][:], AF.Identity,
                                         bias=bq_t[:, m:m + 1], scale=ISCALE)

            # -- logits + exp + softmax sum --
            with tc.tile_pool(name=f"lg_ps{qb}", bufs=4, space="PSUM") as php, \
                 tc.tile_pool(name=f"s_ps{qb}", bufs=1, space="PSUM") as sphp:
                s_ps = sphp.tile([P, QB], f32, name=f"sps{qb}")
                for kb in range(KT):
                    ps = php.tile([P, QB], f32, tag="ps", name=f"lgps{qb}_{kb}")
                    for e in range(ET):
                        mm(ps[:], kp[:, e, kb * P:(kb + 1) * P], qp[:, e, :],
                           e == 0, e == ET - 1)
                    nc.scalar.activation(expT[:, kb, :], ps[:], AF.Exp,
                                         bias=mask_t[:, kb:kb + 1])
                    mm(s_ps[:], ones_t[:], expT[:, kb, :], kb == 0, kb == KT - 1)
                nc.vector.reciprocal(recip_ts[qb][:], s_ps[:])

            qp_cm.__exit__(None, None, None)  # qp dead after logits

            # -- ctx accumulation --
            ctx_ps_cm = tc.tile_pool(name=f"ctx_ps{qb}", bufs=1, space="PSUM")
            ctx_php = ctx_ps_cm.__enter__()
            cps = [ctx_php.tile([P, QB], f32, tag=f"ps{e}", name=f"ctxps{qb}_{e}")
                   for e in range(ET)]
            for e in range(ET):
                for kb in range(KT):
                    mm(cps[e][:], vp[:, kb, e * P:(e + 1) * P], expT[:, kb, :],
                       kb == 0, kb == KT - 1)

            exp_cm.__exit__(None, None, None)  # expT consumed
            if last_qb:
                kp_cm.__exit__(None, None, None)  # kp dead after last logits

            # -- normalize into SBUF ctx^T --
            ctxs_cm = tc.tile_pool(name=f"ctxs{qb}", bufs=1)
            ctxs_pool = ctxs_cm.__enter__()
            ctx_sb = ctxs_pool.tile([P, ET, QB], f32r, name=f"ctx{qb}")
            for e in range(ET):
                nc.vector.tensor_mul(ctx_sb[:, e, :], cps[e][:], recip_ts[qb][:])
            ctx_ps_cm.__exit__(None, None, None)

            if not last_qb:
                dt_ = dram_pool.tile([P, ET, QB], f32r, name=f"ctxd{qb}")
                for e in range(ET):
                    nc.gpsimd.dma_start(dt_[:, e, :], ctx_sb[:, e, :])
                ctx_bounce.append(dt_)
                ctxs_cm.__exit__(None, None, None)
            else:
                ctx_bounce.append(None)
                ctx_last = ctx_sb
                ctx_last_cm = ctxs_cm

        # ---- out phase: out[q, :] = ctx @ ow + ob (ow streamed small) ----
        with tc.tile_pool(name="ctx_back", bufs=1) as cb, \
             tc.tile_pool(name="ow_s", bufs=8) as ows, \
             tc.tile_pool(name="outsb", bufs=6) as osb, \
             tc.tile_pool(name="out_ps", bufs=1, space="PSUM") as php:
            ob_t = cb.tile([P, D], f32, name="ob_t")
            nc.sync.dma_start(ob_t[:], ob_bc)
            ctx_ts = []
            for qb in range(NQB):
                if ctx_bounce[qb] is not None:
                    ctx_t = cb.tile([P, ET, QB], f32r, tag=f"cback{qb}",
                                    name=f"cb{qb}")
                    for e in range(ET):
                        nc.gpsimd.dma_start(ctx_t[:, e, :], ctx_bounce[qb][:, e, :])
                    ctx_ts.append(ctx_t)
                else:
                    ctx_ts.append(ctx_last)
            MQ = QB // P
            for nd in range(D // DNB):
                pss = {}
                for qb in range(NQB):
                    for mq in range(MQ):
                        pss[(qb, mq)] = php.tile(
                            [P, DNB], f32, tag=f"ps{qb}_{mq}",
                            name=f"ops{nd}_{qb}_{mq}")
                for e in range(ET):
                    ow_t = ows.tile([P, DNB], f32r, tag="ow_s",
                                    name=f"ow{nd}_{e}")
                    nc.sync.dma_start(ow_t[:],
                                      ow_r[:, e, nd * DNB:(nd + 1) * DNB])
                    for qb in range(NQB):
                        for mq in range(MQ):
                            mm(pss[(qb, mq)][:],
                               ctx_ts[qb][:, e, mq * P:(mq + 1) * P],
                               ow_t[:], e == 0, e == ET - 1)
                for qb in range(NQB):
                    for mq in range(MQ):
                        ot = osb.tile([P, DNB], f32, tag="ot",
                                      name=f"ot{nd}_{qb}_{mq}")
                        nc.vector.tensor_add(ot[:], pss[(qb, mq)][:],
                                             ob_t[:, nd * DNB:(nd + 1) * DNB])
                        nc.gpsimd.dma_start(
                            out[qb * QB + mq * P: qb * QB + (mq + 1) * P,
                                nd * DNB:(nd + 1) * DNB], ot[:])

        if ctx_last_cm is not None:
            ctx_last_cm.__exit__(None, None, None)
        vp_cm.__exit__(None, None, None)
        dram_cm.__exit__(None, None, None)
        smalls_cm.__exit__(None, None, None)

    nc.compile()
    return nc


def make_in_maps(v, k, q, mask, wq_w, wq_b, wk_w, wk_b, wv_w, wv_b, out_w, out_b,
                 n_cores=8, D=1024, E=1024, SK=2048, QSH=1024):
    """Host-side shard + layout prep (pure data movement, no math)."""
    ET = E // P
    KT = SK // P
    f = np.float32
    wq_w = np.ascontiguousarray(np.asarray(wq_w, f))
    wk_w = np.ascontiguousarray(np.asarray(wk_w, f))
    wv_w = np.ascontiguousarray(np.asarray(wv_w, f))
    out_w = np.ascontiguousarray(np.asarray(out_w, f))
    bq_col = np.ascontiguousarray(np.asarray(wq_b, f).reshape(ET, P).T)
    bk_col = np.ascontiguousarray(np.asarray(wk_b, f).reshape(ET, P).T)
    bv_bc = np.ascontiguousarray(np.broadcast_to(np.asarray(wv_b, f), (P, E)))
    ob_bc = np.ascontiguousarray(
        np.broadcast_to(np.asarray(out_b, f), (P, len(out_b))))
    ones_arr = np.ones((P, P), f)
    in_maps = []
    for c in range(n_cores):
        b, h = divmod(c, 2)
        qTc = np.ascontiguousarray(np.asarray(q[b, h * QSH:(h + 1) * QSH, :], f).T)
        kTc = np.ascontiguousarray(np.asarray(k[b], f).T)
        vTc = np.ascontiguousarray(np.asarray(v[b], f).T)
        mc = np.ascontiguousarray(np.asarray(mask[b, 0], f).reshape(KT, P).T)
        in_maps.append(dict(qT=qTc, kT=kTc, vT=vTc, mask_cols=mc,
                            ones_d=ones_arr,
                            wq=wq_w, wk=wk_w, wv=wv_w, ow=out_w,
                            bq_col=bq_col, bk_col=bk_col,
                            bv_bc=bv_bc, ob_bc=ob_bc))
    return in_maps


_NC_CACHE = {}


def kernel(v, k, q, mask, wq_w, wq_b, wk_w, wk_b, wv_w, wv_b, out_w, out_b):
    from concourse.bass_utils import run_bass_kernel_spmd

    B, S, D = 4, 2048, 1024
    E, QSH = 1024, 1024
    if "nc" not in _NC_CACHE:
        _NC_CACHE["nc"] = build_nc(D=D, E=E, SK=S, QSH=QSH, QB=512)
    nc = _NC_CACHE["nc"]

    in_maps = make_in_maps(v, k, q, mask, wq_w, wq_b, wk_w, wk_b, wv_w, wv_b,
                           out_w, out_b, n_cores=8, D=D, E=E, SK=S, QSH=QSH)
    trace = bool(int(os.environ.get("BASS_KERNEL_TRACE", "0")))
    res = run_bass_kernel_spmd(nc, in_maps, core_ids=list(range(8)), trace=trace)
    if trace:
        print(f"HW exec time: {res.exec_time_ns} ns")
        _NC_CACHE["last_exec_time_ns"] = res.exec_time_ns
        _NC_CACHE["last_trace"] = res.instructions_and_trace

    outp = np.empty((B, S, D), np.float32)
    for c in range(8):
        b, h = divmod(c, 2)
        outp[b, h * QSH:(h + 1) * QSH, :] = res.results[c]["out"]
    return outp



# revision 2
# speedup vs baseline: 1.0016x; 1.0016x over previous
"""Single-head attention (B=4, S=2048, D=E=1024) on 8 trn2 NeuronCores.

Sharding: data-parallel over (batch, q-half) -> 8 shards. Each core: 1024-row
q shard + full 2048 keys of its batch (K/V projections recomputed on both
cores of a batch pair; zero collectives).

Layout: all matmul operands bf16 (host-cast) -> halves DMA traffic and
lets kp/vp/qp/ow live in SBUF simultaneously (no re-streaming, no DRAM
bounce).  Weight tiles are per-t (fine-grained deps) in two alternating
pools (wq->wv, wk->ow) so the next phase's weights prefetch during the
previous phase without SBUF aliasing stalls.  Output-tile-outer loops with
rotating PSUM banks keep drain engines (ACT/DVE) pipelined behind the PE;
softmax sums accumulate on DVE (one ones-matmul per q-block); ctx
accumulation is split into half-key groups so it never waits for the last
exp.  Warmup matmuls keep the PE p-state ramp off the critical path.

Per-core math (all "T" tensors token-transposed on host):
  qp^T [E,q]  = (lhsT=wq[D,E], rhs=qT[D,q]) * (1/sqrt E) + bq/sqrt(E)
  kp^T [E,k]  = (lhsT=wk, rhs=kT) + bk
  vp   [k,E]  = (lhsT=vT[D,k], rhs=wv[D,E]) + bv
  lgT  [k,q]  = (lhsT=kp slice, rhs=qp slice)        (scale folded into qp)
  expT [k,q]  = Exp(lgT + mask*NEG)                  (ACT, per-partition bias)
  s    [.,q]  = ones-matmul over DVE-accumulated exp sums
  ctx^T[E,q]  = (lhsT=vp slice, rhs=expT) * recip(s)
  out  [q,D]  = (lhsT=ctx^T slice, rhs=ow[E,D]) + ob
"""

import os
import numpy as np

P = 128
NEG = -1.0e9
N_WARM = 20


def build_nc(D=1024, E=1024, SK=2048, QSH=1024, QB=512):
    import concourse.bass as bass
    import concourse.mybir as mybir
    import concourse.tile as tile
    from concourse import bacc

    f32 = mybir.dt.float32
    f32r = mybir.dt.float32r
    bf16 = mybir.dt.bfloat16
    AF = mybir.ActivationFunctionType

    DT = D // P           # contraction tiles over model dim (8)
    ET = E // P           # enc tiles (8)
    KT = SK // P          # key tiles (16)
    NQB = QSH // QB       # q blocks (2)
    NKB = SK // QB        # key blocks of 512 (4)
    MQ = QB // P          # q subtiles per block (4)
    ND = D // QB          # out col blocks (2)
    HKT = KT // 2         # key tiles per half (8)
    ISCALE = 1.0 / float(np.sqrt(E))

    nc = bacc.Bacc(trn_type="TRN2")

    # ---- I/O ----
    qT = nc.dram_tensor("qT", [D, QSH], bf16, kind="ExternalInput")[:, :]
    kT = nc.dram_tensor("kT", [D, SK], bf16, kind="ExternalInput")[:, :]
    vT = nc.dram_tensor("vT", [D, SK], bf16, kind="ExternalInput")[:, :]
    wq = nc.dram_tensor("wq", [D, E], bf16, kind="ExternalInput")[:, :]
    wk = nc.dram_tensor("wk", [D, E], bf16, kind="ExternalInput")[:, :]
    wv = nc.dram_tensor("wv", [D, E], bf16, kind="ExternalInput")[:, :]
    ow = nc.dram_tensor("ow", [E, D], bf16, kind="ExternalInput")[:, :]
    mask_cols = nc.dram_tensor("mask_cols", [P, KT], f32, kind="ExternalInput")[:, :]
    ones_d = nc.dram_tensor("ones_d", [P, P], f32r, kind="ExternalInput")[:, :]
    bq_col = nc.dram_tensor("bq_col", [P, ET], f32, kind="ExternalInput")[:, :]
    bk_col = nc.dram_tensor("bk_col", [P, ET], f32, kind="ExternalInput")[:, :]
    bv_bc = nc.dram_tensor("bv_bc", [P, E], f32, kind="ExternalInput")[:, :]
    ob_bc = nc.dram_tensor("ob_bc", [P, D], f32, kind="ExternalInput")[:, :]
    out = nc.dram_tensor("out", [QSH, D], f32, kind="ExternalOutput")[:, :]

    qT_r = qT.rearrange("(t p) n -> p t n", p=P)   # [128, DT, QSH]
    kT_r = kT.rearrange("(t p) n -> p t n", p=P)
    vT_r = vT.rearrange("(t p) n -> p t n", p=P)
    wq_r = wq.rearrange("(t p) n -> p t n", p=P)   # [128, DT, E]
    wk_r = wk.rearrange("(t p) n -> p t n", p=P)
    wv_r = wv.rearrange("(t p) n -> p t n", p=P)
    ow_r = ow.rearrange("(t p) n -> p t n", p=P)   # [128, ET, D]

    def mm(ps, lhsT, rhs, start, stop):
        nc.tensor.matmul(ps, lhsT, rhs, start=start, stop=stop)

    with tile.TileContext(nc) as tc:
        # ---- persistent pools (entered in reverse order of death; LIFO) ----
        smalls_cm = tc.tile_pool(name="smalls", bufs=1)
        smalls = smalls_cm.__enter__()
        osb_cm = tc.tile_pool(name="osb", bufs=3)
        osb = osb_cm.__enter__()
        ctxp_cm = tc.tile_pool(name="ctxp", bufs=2)
        ctxp = ctxp_cm.__enter__()
        expp_cm = tc.tile_pool(name="expp", bufs=1)
        expp = expp_cm.__enter__()
        vpp_cm = tc.tile_pool(name="vpp", bufs=1)
        vpp = vpp_cm.__enter__()
        kpp_cm = tc.tile_pool(name="kpp", bufs=1)
        kpp = kpp_cm.__enter__()
        qpp_cm = tc.tile_pool(name="qpp", bufs=1)
        qpp = qpp_cm.__enter__()
        # alternating weight pools: wA holds wq then wv, wB holds wk then ow.
        wA_cm = tc.tile_pool(name="wA", bufs=1)
        wA = wA_cm.__enter__()
        wB_cm = tc.tile_pool(name="wB", bufs=1)
        wB = wB_cm.__enter__()
        # stream pools: S_A holds qT blocks then vT blocks, S_B holds kT.
        sA_cm = tc.tile_pool(name="sA", bufs=4)
        sA = sA_cm.__enter__()
        sB_cm = tc.tile_pool(name="sB", bufs=4)
        sB = sB_cm.__enter__()

        # one global PSUM pool: 8 bank tags, allocation order chosen so each
        # phase's first bank aliases the earliest-released predecessor.
        gps_cm = tc.tile_pool(name="gps", bufs=1, space="PSUM")
        gps = gps_cm.__enter__()

        # smalls on gpsimd SWDGE: keeps the HWDGE free for the wq stream.
        # Issued after the first qT blocks (Pool program order).
        qp = qpp.tile([P, ET, QSH], bf16, name="qp")
        kp = kpp.tile([P, ET, SK], bf16, name="kp")
        vp = vpp.tile([P, KT, E], bf16, name="vp")

        # wq per-t tiles (fine-grained: QP matmuls start on first arrival);
        # t0 lands as two separate half-tiles so the very first matmuls
        # only wait for a 128KB transfer
        wq_ts = []
        qt01 = sA.tile([P, 4, QB], bf16, tag="s4", name="qt01")
        wq0a = wA.tile([P, E // 2], bf16, tag="w0a", name="wq0a")
        nc.sync.dma_start(wq0a[:], wq_r[:, 0, :E // 2])
        wq0b = wA.tile([P, E // 2], bf16, tag="w0b", name="wq0b")
        nc.sync.dma_start(wq0b[:], wq_r[:, 0, E // 2:])
        wq_ts.append((wq0a, wq0b))
        for t in range(1, DT):
            w = wA.tile([P, E], bf16, tag=f"w{t}", name=f"wq{t}")
            nc.sync.dma_start(w[:], wq_r[:, t, :])
            wq_ts.append(w)
            if t == 3:
                nc.sync.dma_start(qt01[:], qT_r[:, 4:8, 0:QB])
        wk_ts = []
        for t in range(DT):
            w = wB.tile([P, E], bf16, tag=f"w{t}", name=f"wk{t}")
            nc.sync.dma_start(w[:], wk_r[:, t, :])
            wk_ts.append(w)
        # qh0 tb0 as per-t tiles (earliest possible first matmul), rest blocks
        qt0_ts = []
        for ti in range(4):
            qt = sA.tile([P, QB], bf16, tag="s", name=f"qt00_{ti}")
            nc.gpsimd.dma_start(qt[:], qT_r[:, ti, 0:QB])
            qt0_ts.append(qt)
        bq_t = smalls.tile([P, ET], f32, name="bqc")
        nc.gpsimd.dma_start(bq_t[:], bq_col)
        qt1_bs = []
        for tb in range(2):
            qb_t = sA.tile([P, 4, QB], bf16, tag="s4", name=f"qt1_{tb}")
            nc.gpsimd.dma_start(qb_t[:], qT_r[:, 4 * tb:4 * tb + 4, QB:2 * QB])
            qt1_bs.append(qb_t)
        ones_t = smalls.tile([P, P], f32r, name="ones")
        nc.gpsimd.dma_start(ones_t[:], ones_d)
        mask_t = smalls.tile([P, KT], f32, name="maskc")
        nc.gpsimd.dma_start(mask_t[:], mask_cols)
        bk_t = smalls.tile([P, ET], f32, name="bkc")
        nc.gpsimd.dma_start(bk_t[:], bk_col)
        bv_t = smalls.tile([P, E], f32, name="bvc")
        nc.gpsimd.dma_start(bv_t[:], bv_bc)
        ob_t = smalls.tile([P, D], f32, name="obc")
        nc.gpsimd.dma_start(ob_t[:], ob_bc)
        recip_ts = [smalls.tile([P, QB], f32, name=f"recip{i}") for i in range(NQB)]
        sacc_ts = [smalls.tile([P, QB], f32r, name=f"sacc{i}") for i in range(NQB)]

        # ---- warmup: keep the PE p-state ramp off the critical path ----
        with tc.tile_pool(name="warm", bufs=1) as wrm:
            dummy = wrm.tile([P, P], bf16, name="dummy")
            nc.vector.memset(dummy[:], 0.0)
            wps = gps.tile([P, P], f32, tag="b7", name="wps")
            for _ in range(N_WARM):
                mm(wps[:], dummy[:], dummy[:], True, True)

        # ---- QP: qp^T [E, QSH]; loop orders matched to DMA arrival ----
        # qh0 t0-3: t-outer (each arriving wq tile unlocks 8 matmuls);
        # qh0 t4-6: t-outer; t7: m-outer with stops so drains pipeline.
        def wq_sl(t, m):
            if t == 0:
                half = wq_ts[0][0] if m < 4 else wq_ts[0][1]
                return half[:, (m % 4) * P:(m % 4 + 1) * P]
            return wq_ts[t][:, m * P:(m + 1) * P]

        pss0 = [gps.tile([P, QB], f32, tag=f"b{m}", name=f"qpps0_{m}")
                for m in range(ET)]
        for ti in range(4):
            for m in range(ET):
                mm(pss0[m][:], wq_sl(ti, m), qt0_ts[ti][:], ti == 0, False)
        for ti in range(3):
            t = 4 + ti
            for m in range(ET):
                mm(pss0[m][:], wq_sl(t, m), qt01[:, ti, :], False, False)
        for m in range(ET):
            mm(pss0[m][:], wq_sl(7, m), qt01[:, 3, :], False, True)
            nc.scalar.activation(qp[:, m, 0:QB], pss0[m][:], AF.Identity,
                                 bias=bq_t[:, m:m + 1], scale=ISCALE)
        # qh1: m-outer chains (all data resident), banks reused as drained
        for m in range(ET):
            ps = gps.tile([P, QB], f32, tag=f"b{m}", name=f"qpps1_{m}")
            for t in range(DT):
                mm(ps[:], wq_sl(t, m),
                   qt1_bs[t // 4][:, t % 4, :], t == 0, t == DT - 1)
            nc.scalar.activation(qp[:, m, QB:2 * QB], ps[:], AF.Identity,
                                 bias=bq_t[:, m:m + 1], scale=ISCALE)

        # ---- KP: kp^T [E, SK] (kT streamed per key block; m-outer chains) ----
        kcnt = 0
        for nb in range(NKB):
            kt_bs = []
            for tb in range(2):
                kt_b = sB.tile([P, 4, QB], bf16, tag="s", name=f"kt{nb}_{tb}")
                nc.gpsimd.dma_start(
                    kt_b[:], kT_r[:, 4 * tb:4 * tb + 4, nb * QB:(nb + 1) * QB])
                kt_bs.append(kt_b)
            for m in range(ET):
                ps = gps.tile([P, QB], f32, tag=f"b{kcnt % 3}",
                              name=f"kpps{nb}_{m}")
                kcnt += 1
                for t in range(DT):
                    mm(ps[:], wk_ts[t][:, m * P:(m + 1) * P],
                       kt_bs[t // 4][:, t % 4, :], t == 0, t == DT - 1)
                nc.scalar.activation(kp[:, m, nb * QB:(nb + 1) * QB],
                                     ps[:], AF.Identity,
                                     bias=bk_t[:, m:m + 1])

        # wv reuses wA slots (freed as QP's last reads complete); prefetches
        # during KP.  ow reuses wB slots; prefetches during VP.
        wv0a = wA.tile([P, E // 2], bf16, tag="w0a", name="wv0a")
        nc.sync.dma_start(wv0a[:], wv_r[:, 0, :E // 2])
        wv0b = wA.tile([P, E // 2], bf16, tag="w0b", name="wv0b")
        nc.sync.dma_start(wv0b[:], wv_r[:, 0, E // 2:])
        wv_ts = [(wv0a, wv0b)]
        for t in range(1, DT):
            w = wA.tile([P, E], bf16, tag=f"w{t}", name=f"wv{t}")
            nc.sync.dma_start(w[:], wv_r[:, t, :])
            wv_ts.append(w)

        def wv_sl(t, h):
            if t == 0:
                return wv_ts[0][h][:]
            return wv_ts[t][:, h * QB:(h + 1) * QB]

        # ---- VP: vp [SK, E] (vT streamed per key block; tile-outer chains) ----
        vcnt = 0
        for nb in range(NKB):
            vt_bs = []
            for tb in range(2):
                vt_b = sA.tile([P, 4, QB], bf16, tag="s4", name=f"vt{nb}_{tb}")
                nc.sync.dma_start(
                    vt_b[:], vT_r[:, 4 * tb:4 * tb + 4, nb * QB:(nb + 1) * QB])
                vt_bs.append(vt_b)
            for kbl in range(MQ):
                kb = nb * MQ + kbl
                for h in range(2):
                    ps = gps.tile([P, QB], f32, tag=f"b{3 + vcnt % 3}",
                                  name=f"vpps{kb}_{h}")
                    vcnt += 1
                    for t in range(DT):
                        mm(ps[:], vt_bs[t // 4][:, t % 4, kbl * P:(kbl + 1) * P],
                           wv_sl(t, h), t == 0, t == DT - 1)
                    nc.vector.tensor_add(vp[:, kb, h * QB:(h + 1) * QB],
                                         ps[:], bv_t[:, h * QB:(h + 1) * QB])

        ow_ts = []
        for e in range(ET):
            w = wB.tile([P, D], bf16, tag=f"w{e}", name=f"ow{e}")
            nc.sync.dma_start(w[:], ow_r[:, e, :])
            ow_ts.append(w)

        # ---- per q-block: logits/exp -> ctx -> out ----
        for qb in range(NQB):
            q0 = qb * QB
            expa = expp.tile([P, HKT, QB], bf16, tag="expa", name=f"expa{qb}")
            expb = expp.tile([P, HKT, QB], bf16, tag="expb", name=f"expb{qb}")
            exp_half = [expa, expb]
            # ctx in two halves so the out phase never waits for the last
            # normalize (whole-tile dependency granularity)
            ctxa = ctxp.tile([P, 4, QB], bf16, tag="ctxa", name=f"ctxa{qb}")
            ctxb = ctxp.tile([P, 4, QB], bf16, tag="ctxb", name=f"ctxb{qb}")
            sacc = sacc_ts[qb]

            # logits + exp; DVE accumulates softmax sums
            for kb in range(KT):
                ps = gps.tile([P, QB], f32, tag=f"b{kb % 3}", name=f"lg{qb}_{kb}")
                for e in range(ET):
                    mm(ps[:], kp[:, e, kb * P:(kb + 1) * P],
                       qp[:, e, q0:q0 + QB], e == 0, e == ET - 1)
                ex = exp_half[kb // HKT]
                nc.scalar.activation(ex[:, kb % HKT, :], ps[:], AF.Exp,
                                     bias=mask_t[:, kb:kb + 1])
                if kb == 0:
                    nc.vector.tensor_copy(sacc[:], ex[:, 0, :])
                else:
                    nc.vector.tensor_add(sacc[:], sacc[:],
                                         ex[:, kb % HKT, :])

            # ctx accumulation in half-key groups: the first group only
            # needs the first 8 exp tiles, so the PE never waits for the
            # last exp.  e-groups of 4 keep PSUM use at 4 banks (b3-b6).
            cps = {}
            for eg in range(2):
                es = range(4 * eg, 4 * eg + 4)
                for half in range(2):
                    ex = exp_half[half]
                    for e in es:
                        if half == 0:
                            cps[e] = gps.tile([P, QB], f32, tag=f"b{3 + e % 4}",
                                              name=f"c{qb}_{e}")
                        for ki in range(HKT):
                            kb = half * HKT + ki
                            mm(cps[e][:], vp[:, kb, e * P:(e + 1) * P],
                               ex[:, ki, :], kb == 0, kb == KT - 1)
                    if eg == 0 and half == 0:
                        # softmax denominator: ones-matmul + reciprocal
                        s_ps = gps.tile([P, QB], f32, tag="b7",
                                        name=f"sps{qb}")
                        mm(s_ps[:], ones_t[:], sacc[:], True, True)
                        nc.vector.reciprocal(recip_ts[qb][:], s_ps[:])
                    if half == 1:
                        for e in es:
                            ctx_half = ctxa if e < 4 else ctxb
                            nc.vector.tensor_mul(ctx_half[:, e % 4, :],
                                                 cps[e][:], recip_ts[qb][:])

            # out block: out[q0:q0+QB, :] = ctx^T.T @ ow + ob
            last_qb = qb == NQB - 1
            ocnt = 0
            for mq in range(MQ):
                for nd in range(ND):
                    rows = slice(q0 + mq * P, q0 + (mq + 1) * P)
                    last_tile = last_qb and mq == MQ - 1 and nd == ND - 1
                    if last_tile:
                        # final tile as two N=256 chains: the first half's
                        # store overlaps the second half's matmuls, so the
                        # kernel tail is one short store chain
                        for sub in range(2):
                            c0 = nd * QB + sub * (QB // 2)
                            ps = gps.tile([P, QB // 2], f32,
                                          tag=f"b{ocnt % 3}",
                                          name=f"o{qb}_{mq}_{nd}_{sub}")
                            ocnt += 1
                            for e in range(ET):
                                ctx_half = ctxa if e < 4 else ctxb
                                mm(ps[:],
                                   ctx_half[:, e % 4, mq * P:(mq + 1) * P],
                                   ow_ts[e][:, c0:c0 + QB // 2],
                                   e == 0, e == ET - 1)
                            ot = osb.tile([P, QB // 2], f32, tag="oth",
                                          name=f"ot{qb}_{mq}_{nd}_{sub}")
                            nc.vector.tensor_add(ot[:], ps[:],
                                                 ob_t[:, c0:c0 + QB // 2])
                            eng = nc.gpsimd if sub == 0 else nc.sync
                            eng.dma_start(out[rows, c0:c0 + QB // 2], ot[:])
                        continue
                    ps = gps.tile([P, QB], f32, tag=f"b{ocnt % 3}",
                                  name=f"o{qb}_{mq}_{nd}")
                    ocnt += 1
                    for e in range(ET):
                        ctx_half = ctxa if e < 4 else ctxb
                        mm(ps[:], ctx_half[:, e % 4, mq * P:(mq + 1) * P],
                           ow_ts[e][:, nd * QB:(nd + 1) * QB],
                           e == 0, e == ET - 1)
                    ot = osb.tile([P, QB], f32, tag="ot",
                                  name=f"ot{qb}_{mq}_{nd}")
                    nc.vector.tensor_add(ot[:], ps[:],
                                         ob_t[:, nd * QB:(nd + 1) * QB])
                    nc.gpsimd.dma_start(out[rows, nd * QB:(nd + 1) * QB],
                                        ot[:])

        gps_cm.__exit__(None, None, None)
        sB_cm.__exit__(None, None, None)
        sA_cm.__exit__(None, None, None)
        wB_cm.__exit__(None, None, None)
        wA_cm.__exit__(None, None, None)
        qpp_cm.__exit__(None, None, None)
        kpp_cm.__exit__(None, None, None)
        vpp_cm.__exit__(None, None, None)
        expp_cm.__exit__(None, None, None)
        ctxp_cm.__exit__(None, None, None)
        osb_cm.__exit__(None, None, None)
        smalls_cm.__exit__(None, None, None)

    nc.compile()
    return nc


def make_in_maps(v, k, q, mask, wq_w, wq_b, wk_w, wk_b, wv_w, wv_b, out_w, out_b,
                 n_cores=8, D=1024, E=1024, SK=2048, QSH=1024):
    """Host-side shard + layout prep (pure data movement + dtype cast)."""
    import ml_dtypes
    bf = ml_dtypes.bfloat16
    ET = E // P
    KT = SK // P
    f = np.float32
    iscale = f(1.0 / np.sqrt(E))
    wq_bf = np.ascontiguousarray(np.asarray(wq_w, f).astype(bf))
    wk_bf = np.ascontiguousarray(np.asarray(wk_w, f).astype(bf))
    wv_bf = np.ascontiguousarray(np.asarray(wv_w, f).astype(bf))
    ow_bf = np.ascontiguousarray(np.asarray(out_w, f).astype(bf))
    bq_col = np.ascontiguousarray((np.asarray(wq_b, f) * iscale).reshape(ET, P).T)
    bk_col = np.ascontiguousarray(np.asarray(wk_b, f).reshape(ET, P).T)
    bv_bc = np.ascontiguousarray(np.broadcast_to(np.asarray(wv_b, f), (P, E)))
    ob_bc = np.ascontiguousarray(
        np.broadcast_to(np.asarray(out_b, f), (P, len(out_b))))
    ones_arr = np.ones((P, P), f)
    in_maps = []
    for c in range(n_cores):
        b, h = divmod(c, 2)
        qTc = np.ascontiguousarray(
            np.asarray(q[b, h * QSH:(h + 1) * QSH, :], f).T.astype(bf))
        kTc = np.ascontiguousarray(np.asarray(k[b], f).T.astype(bf))
        vTc = np.ascontiguousarray(np.asarray(v[b], f).T.astype(bf))
        mc = np.ascontiguousarray(
            (np.asarray(mask[b, 0], f) * np.float32(NEG)).reshape(KT, P).T)
        in_maps.append(dict(qT=qTc, kT=kTc, vT=vTc, mask_cols=mc,
                            ones_d=ones_arr,
                            wq=wq_bf, wk=wk_bf, wv=wv_bf, ow=ow_bf,
                            bq_col=bq_col, bk_col=bk_col,
                            bv_bc=bv_bc, ob_bc=ob_bc))
    return in_maps


_NC_CACHE = {}


def kernel(v, k, q, mask, wq_w, wq_b, wk_w, wk_b, wv_w, wv_b, out_w, out_b):
    from concourse.bass_utils import run_bass_kernel_spmd

    B, S, D = 4, 2048, 1024
    E, QSH = 1024, 1024
    if "nc" not in _NC_CACHE:
        _NC_CACHE["nc"] = build_nc(D=D, E=E, SK=S, QSH=QSH, QB=512)
    nc = _NC_CACHE["nc"]

    in_maps = make_in_maps(v, k, q, mask, wq_w, wq_b, wk_w, wk_b, wv_w, wv_b,
                           out_w, out_b, n_cores=8, D=D, E=E, SK=S, QSH=QSH)
    trace = bool(int(os.environ.get("BASS_KERNEL_TRACE", "0")))
    res = run_bass_kernel_spmd(nc, in_maps, core_ids=list(range(8)), trace=trace)
    if trace:
        print(f"HW exec time: {res.exec_time_ns} ns")
        _NC_CACHE["last_exec_time_ns"] = res.exec_time_ns
        _NC_CACHE["last_trace"] = res.instructions_and_trace

    outp = np.empty((B, S, D), np.float32)
    for c in range(8):
        b, h = divmod(c, 2)
        outp[b, h * QSH:(h + 1) * QSH, :] = res.results[c]["out"]
    return outp
